# revision 1
# baseline (speedup 1.0000x reference)
"""Trainium2 Bass kernel v3: batched causal attention (B=4, S=4096, E=256, f32).

Sharding: 2 cores per batch element; QUERY chunks split within the pair
(even core gets 512-row chunks {7,5,2,0}, odd {6,4,3,1}) so causal work is
perfectly balanced with NO cross-core communication.  Both cores hold full
K/V for their batch.  SPMD-uniform instruction stream: 4 slots with padded
k-tile counts (32,24,16,8); per-core DATA (exp-bias table, mask table)
kills padding tiles and applies causal masks.

Per-core dataflow (bf16 matmuls, f32 PSUM):
  phase1: X/Z cast-loaded (gpsimd DMA), X^T/Z^T via PE transposes,
  Q^T=(WqT@X^T + bq)/16 (scalar act), K^T=WkT@Z^T.
  attention: per k-tile pair, S^T = K^T(stat).Q^T accumulated with a
  causal-mask matmul (-1e9*I stationary x {0,1} fp8 pattern) for the last
  4 pairs of each slot; exp on scalar engine (bias 0 valid/-1e30 pad)
  -> P^T bf16; O'^T += Z(stat)@P^T (V projection deferred); rowsum via DVE
  pair-sums + gpsimd accumulation.
  post (per slot, overlapped with next slot): rowsum reduced by ones-matmul,
  reciprocal on DVE, O = O'@Wv^T + bv*rowsum (rank-1 matmul) in natural
  layout (stationary = O'^T slices, no output transposes), scaled by
  1/rowsum, DMA'd out.
"""

import numpy as np

B = 4
S = 4096
E = 256
NSLOT = 4
PADN = (32, 24, 16, 8)       # padded k-tiles per slot
CHUNKS = ((7, 5, 2, 0), (6, 4, 3, 1))   # slot -> 512-chunk, per parity
NPAIR = sum(p // 2 for p in PADN)        # 40 exp pairs
NMASK = 4                    # masked pairs per slot (last 4)

_COMPILED = {}


def _build():
    import concourse.bass as bass
    import concourse.tile as tile
    from concourse import mybir, bacc
    from concourse.masks import make_identity

    f32 = mybir.dt.float32
    bf16 = mybir.dt.bfloat16
    fp8 = mybir.dt.float8e4
    Exp = mybir.ActivationFunctionType.Exp
    Ident = mybir.ActivationFunctionType.Identity
    Add = mybir.AluOpType.add

    nc = bacc.Bacc("TRN2", target_bir_lowering=False, debug=False,
                   enable_asserts=True, num_devices=8)

    x_ext = nc.dram_tensor("x", [2048, E], f32, kind="ExternalInput")
    z_ext = nc.dram_tensor("z", [S, E], f32, kind="ExternalInput")
    wq_ext = nc.dram_tensor("wq2", [128, 2, 256], f32, kind="ExternalInput")
    wk_ext = nc.dram_tensor("wk2", [128, 2, 256], f32, kind="ExternalInput")
    wv_ext = nc.dram_tensor("wv2", [128, 2, 256], f32, kind="ExternalInput")
    bqs_ext = nc.dram_tensor("bqs", [128, 2], f32, kind="ExternalInput")
    bv_ext = nc.dram_tensor("bvr", [E], f32, kind="ExternalInput")
    ebias_ext = nc.dram_tensor("ebias", [NPAIR], f32, kind="ExternalInput")
    masks_ext = nc.dram_tensor("masks", [16, 128, 2, 512], fp8,
                               kind="ExternalInput")
    out_ext = nc.dram_tensor("out", [2048, E], f32, kind="ExternalOutput")

    with tile.TileContext(nc) as tc:
        with tc.tile_pool(name="singles", bufs=1) as singles:
            # ---- constants (identity first: unblocks transposes early) -----
            ident_bf = singles.tile([128, 128], bf16)
            make_identity(nc, ident_bf[:])
            ident_f32 = singles.tile([128, 128], f32)
            make_identity(nc, ident_f32[:])
            ident_big = singles.tile([128, 128], bf16)
            nc.vector.tensor_scalar_mul(ident_big[:], ident_bf[:], -1e9)
            wq2 = singles.tile([128, 2, 256], bf16)
            wk2 = singles.tile([128, 2, 256], bf16)
            wv2 = singles.tile([128, 2, 256], bf16)
            for wt, wext in ((wq2, wq_ext), (wk2, wk_ext), (wv2, wv_ext)):
                nc.gpsimd.dma_start(out=wt[:], in_=wext[:])
            bqs = singles.tile([128, 2], f32)
            nc.sync.dma_start(out=bqs[:], in_=bqs_ext[:])
            _ = None
            bv_sb = singles.tile([1, E], bf16)
            nc.gpsimd.dma_start(out=bv_sb[:], in_=bv_ext.ap().rearrange(
                "(one e) -> one e", one=1))
            ebias = singles.tile([128, NPAIR], f32)
            nc.sync.dma_start(
                out=ebias[:],
                in_=bass.AP(tensor=ebias_ext, offset=0, ap=[[0, 128], [1, NPAIR]]))
            ones_col = singles.tile([128, 1], f32)
            nc.vector.memset(ones_col[:], 1.0)
            ones_full = singles.tile([128, 128], f32)
            nc.vector.memset(ones_full[:], 1.0)

            # ---- big persistent SBUF (per-chunk tiles for fine deps) -------
            z_nat = [singles.tile([128, 4, E], bf16, name=f"z_nat{i}")
                     for i in range(8)]
            kT2 = [singles.tile([128, 2, 512], bf16, name=f"kT2_{i}")
                   for i in range(8)]
            qT2 = [singles.tile([128, 2, 512], bf16, name=f"qT2_{i}")
                   for i in range(4)]
            xnat_f = [singles.tile([128, 4, E], f32, name=f"xnf{i}")
                      for i in range(4)]
            maskt = singles.tile([128, 16, 2, 512], fp8)

            with tc.tile_pool(name="ps_s", bufs=2, space="PSUM") as ps_s, \
                 tc.tile_pool(name="pTp", bufs=6) as pTp, \
                 tc.tile_pool(name="rsp", bufs=4) as rsp, \
                 tc.tile_pool(name="postp", bufs=2) as postp, \
                 tc.tile_pool(name="trsb", bufs=4) as trsb:

                # ---- phase 1: transposes + projections ---------------------
                def kproj(sc, zT):
                    psk = ps_s.tile([128, 2, 512], f32, tag="pss", name="psk")
                    for ft in range(2):
                        for eh in range(2):
                            nc.tensor.matmul(
                                psk[:, ft, :], wk2[:, eh, 128 * ft:128 * (ft + 1)],
                                zT[:, eh, :],
                                start=(eh == 0), stop=(eh == 1))
                    nc.vector.tensor_copy(out=kT2[sc][:], in_=psk[:])

                def qproj(s, xT):
                    psq = ps_s.tile([128, 2, 512], f32, tag="pss", name="psq")
                    for ft in range(2):
                        for eh in range(2):
                            nc.tensor.matmul(
                                psq[:, ft, :],
                                wq2[:, eh, 128 * ft:128 * (ft + 1)],
                                xT[:, eh, :],
                                start=(eh == 0), stop=(eh == 1))
                    for ft in range(2):
                        nc.vector.tensor_scalar(
                            out=qT2[s][:, ft, :],
                            in0=psq[:, ft, :],
                            scalar1=1.0 / 16.0, scalar2=bqs[:, ft:ft + 1],
                            op0=mybir.AluOpType.mult, op1=Add)

                # ---- attention (phase 1 interleaved into slot 0) -----------
                with tc.tile_pool(name="ps_o", bufs=1, space="PSUM") as ps_o:

                    gp = 0
                    gm = 0
                    post_queue = []

                    def post_slot(s, pso, rsacc):
                        psr = ps_p.tile([128, 512], f32, tag="psp", name="psr")
                        nc.tensor.matmul(psr[:, :], ones_full[:], rsacc[:],
                                         start=True, stop=True)
                        rs_sb = rsp.tile([128, 512], f32, tag="rs_sb",
                                         name="rs_sb")
                        nc.vector.tensor_copy(out=rs_sb[:], in_=psr[:])
                        rs_row2 = rsp.tile([1, 512], bf16, tag="rs_row2",
                                           name="rs_row2")
                        nc.scalar.copy(out=rs_row2[:], in_=psr[0:1, :])
                        psT = ps_p.tile([128, 4, 128], f32, tag="psp",
                                        name="psT")
                        for t in range(4):
                            nc.tensor.transpose(psT[:, t, :],
                                                rs_sb[:, 128 * t:128 * (t + 1)],
                                                ident_f32[:])
                        rs_t = rsp.tile([128, 4], f32, tag="rs_t", name="rs_t")
                        nc.vector.reciprocal(out=rs_t[:], in_=psT[:, :, 0])
                        po_sb = postp.tile([128, 2, 512], bf16, tag="po_sb",
                                           name="po_sb")
                        nc.scalar.copy(out=po_sb[:, 0, :], in_=pso[:, 0, :])
                        nc.vector.tensor_copy(out=po_sb[:, 1, :],
                                              in_=pso[:, 1, :])
                        obuf = postp.tile([128, 4, E], f32, tag="obuf",
                                          name="obuf")
                        for t in range(4):
                            pso3 = ps_p.tile([128, E], f32, tag="psp",
                                             name="pso3",
                                             padded_shape=[128, 512])
                            for eh in range(2):
                                nc.tensor.matmul(
                                    pso3[:], po_sb[:, eh, 128 * t:128 * (t + 1)],
                                    wv2[:, eh, :], start=(eh == 0), stop=False,
                                    skip_group_check=True)
                            nc.tensor.matmul(
                                pso3[:], rs_row2[0:1, 128 * t:128 * (t + 1)],
                                bv_sb[:], start=False, stop=True,
                                skip_group_check=True)
                            nc.vector.tensor_scalar_mul(obuf[:, t, :], pso3[:],
                                                        rs_t[:, t:t + 1])
                        nc.sync.dma_start(
                            out=out_ext[512 * s:512 * (s + 1), :].rearrange(
                                "(t p) e -> p t e", p=128),
                            in_=obuf[:])

                    def emit_scores(s, p, npair):
                        nonlocal gp, gm
                        masked = p >= npair - NMASK
                        pss = ps_s.tile([128, 2, 512], f32, tag="pss",
                                        name="pss")
                        for i in range(2):
                            ll = 2 * p + i
                            for fh in range(2):
                                nc.tensor.matmul(
                                    pss[:, i, :],
                                    kT2[ll // 4][:, fh,
                                                 128 * (ll % 4):128 * (ll % 4 + 1)],
                                    qT2[s][:, fh, :],
                                    start=(fh == 0),
                                    stop=(fh == 1) and not masked)
                            if masked:
                                nc.tensor.matmul(
                                    pss[:, i, :], ident_big[:],
                                    maskt[:, gm, i, :],
                                    start=False, stop=True)
                        if masked:
                            gm += 1
                        pT = pTp.tile([128, 2, 512], bf16, tag="pT", name="pT")
                        nc.scalar.activation(out=pT[:], in_=pss[:], func=Exp,
                                             bias=ebias[:, gp:gp + 1],
                                             scale=1.0)
                        gp += 1
                        return pT

                    def emit_pv(s, p, npair, pso, rsacc, pT):
                        for i in range(2):
                            ll = 2 * p + i
                            for eh in range(2):
                                nc.tensor.matmul(
                                    pso[:, eh, :],
                                    z_nat[ll // 4][:, ll % 4,
                                                   128 * eh:128 * (eh + 1)],
                                    pT[:, i, :],
                                    start=(p == 0 and i == 0),
                                    stop=(p == npair - 1 and i == 1),
                                    skip_group_check=True)
                        tmp = rsp.tile([128, 512], bf16, tag="rtmp",
                                       name="rtmp")
                        nc.vector.tensor_tensor(out=tmp[:], in0=pT[:, 0, :],
                                                in1=pT[:, 1, :], op=Add)
                        if p == 0:
                            nc.gpsimd.tensor_copy(out=rsacc[:], in_=tmp[:])
                        else:
                            nc.gpsimd.tensor_tensor(out=rsacc[:], in0=rsacc[:],
                                                    in1=tmp[:], op=Add)

                    pending = []

                    # -- slot 0 with phase-1 interleave --
                    with tc.tile_pool(name="ps_tr", bufs=1, space="PSUM") as ps_tr:
                        def transpose_chunk(nat, sc):
                            f32p = nat.dtype == f32
                            ident = ident_f32 if f32p else ident_bf
                            dst = trsb.tile([128, 2, 512], bf16, tag="tr",
                                            name="tr")
                            for eh in range(2):
                                pst = ps_tr.tile([128, 512],
                                                 f32 if f32p else bf16,
                                                 tag="pstx" if f32p else "pst",
                                                 name="pst")
                                for t in range(4):
                                    nc.tensor.transpose(
                                        pst[:, 128 * t:128 * (t + 1)],
                                        nat[:, t, 128 * eh:128 * (eh + 1)],
                                        ident[:])
                                nc.vector.tensor_copy(out=dst[:, eh, :],
                                                      in_=pst[:])
                            return dst

                        qd0 = [nc.sync, nc.scalar]
                        st = {"zl": 0, "xl": 0, "znl": 0, "mk": False}

                        def load_z(sc):
                            nc.gpsimd.dma_start(
                                out=z_nat[sc][:],
                                in_=z_ext[512 * sc:512 * (sc + 1), :].rearrange(
                                    "(t p) e -> p t e", p=128))

                        def load_x(sc):
                            qd0[0].dma_start(
                                out=xnat_f[sc][:],
                                in_=x_ext[512 * sc:512 * (sc + 1), :].rearrange(
                                    "(t p) e -> p t e", p=128))

                        def prefetch(zc, xc):
                            while st["zl"] < min(zc, 8):
                                load_z(st["zl"]); st["zl"] += 1
                            while st["xl"] < min(xc, 4):
                                load_x(st["xl"]); st["xl"] += 1

                        def zchunk(sc):
                            kproj(sc, transpose_chunk(z_nat[sc], sc))

                        def xchunk(sc):
                            qproj(sc, transpose_chunk(xnat_f[sc], sc))

                        prefetch(2, 1)
                        zchunk(0)
                        xchunk(0)
                        zdone, xdone = 1, 1
                        npair = PADN[0] // 2
                        pso = ps_o.tile([128, 2, 512], f32, tag="pso",
                                        name="pso")
                        rsacc = rsp.tile([128, 512], f32, tag="racc",
                                         name="racc")
                        for p in range(npair):
                            prefetch(zdone + 2, xdone + 1)
                            if p == 6:
                                nc.gpsimd.dma_start(
                                    out=maskt[:],
                                    in_=masks_ext.ap().rearrange(
                                        "m p i f -> p m i f"))
                                st["mk"] = True
                            while zdone < 8 and 4 * zdone < 2 * p + 6:
                                zchunk(zdone)
                                if xdone < 4:
                                    xchunk(xdone)
                                    xdone += 1
                                zdone += 1
                            pT = emit_scores(0, p, npair)
                            pending.append((0, p, npair, pso, rsacc, pT))
                            if len(pending) > 2:
                                emit_pv(*pending.pop(0))
                        prefetch(8, 4)
                        while zdone < 8:
                            zchunk(zdone)
                            zdone += 1
                        while xdone < 4:
                            xchunk(xdone)
                            xdone += 1
                        assert st["mk"]
                        post_queue.append((0, pso, rsacc))

                    # -- slots 1..3 --
                    with tc.tile_pool(name="ps_p", bufs=2, space="PSUM") as ps_p:
                        for s in range(1, NSLOT):
                            npair = PADN[s] // 2
                            pso = ps_o.tile([128, 2, 512], f32, tag="pso",
                                            name="pso")
                            rsacc = rsp.tile([128, 512], f32, tag="racc",
                                             name="racc")
                            for p in range(npair):
                                pT = emit_scores(s, p, npair)
                                pending.append((s, p, npair, pso, rsacc, pT))
                                if len(pending) > 2:
                                    emit_pv(*pending.pop(0))
                                if p == 1 and post_queue:
                                    post_slot(*post_queue.pop())
                            post_queue.append((s, pso, rsacc))
                        while pending:
                            emit_pv(*pending.pop(0))
                        post_slot(*post_queue.pop())

    nc.compile()
    return nc


def _get_nc():
    if "nc" not in _COMPILED:
        _COMPILED["nc"] = _build()
    return _COMPILED["nc"]


def _make_masks():
    """Blocked-region tables per parity: [16 pairs, 128 k, 2, 512 q] in {0,1},
    1 = BLOCKED (gets -1e9 added to the score)."""
    import ml_dtypes
    fp8 = ml_dtypes.float8_e4m3
    ky = np.arange(128)[:, None]
    x = np.arange(512)[None, :]
    diag = [((x < 128 * t + ky)).astype(np.float32) for t in range(4)]
    keepall = np.zeros((128, 512), np.float32)
    blockall = np.ones((128, 512), np.float32)
    res = []
    for par in range(2):
        tiles = []
        for s in range(NSLOT):
            valid = 4 * (CHUNKS[par][s] + 1)
            padded = PADN[s]
            for ll in range(padded - 8, padded):
                if ll >= valid:
                    tiles.append(blockall)   # pad tile (also killed by ebias)
                elif ll >= valid - 4:
                    tiles.append(diag[ll - (valid - 4)])
                else:
                    tiles.append(keepall)
        m = np.stack(tiles).reshape(16, 2, 128, 512).transpose(0, 2, 1, 3)
        res.append(np.ascontiguousarray(m.astype(fp8)))
    return res


def _make_ebias():
    """Exp bias per pair: 0 valid, -1e30 pad; [2][NPAIR] f32."""
    res = []
    for par in range(2):
        vals = []
        for s in range(NSLOT):
            valid = 4 * (CHUNKS[par][s] + 1)
            for p in range(PADN[s] // 2):
                vals.append(0.0 if 2 * p < valid else -1e30)
        res.append(np.asarray(vals, dtype=np.float32))
    return res


def kernel(X, Z, mask, Wq, bq, Wk, bk, Wv, bv):
    X = np.asarray(X, dtype=np.float32)
    Z = np.asarray(Z, dtype=np.float32)
    mask_np = np.asarray(mask)

    causal = bool(np.array_equal(
        mask_np != 0, np.tril(np.ones((S, S), dtype=bool))))
    if not causal:
        return _numpy_ref(X, Z, mask_np, Wq, bq, Wk, bk, Wv, bv)

    from concourse.bass_utils import run_bass_kernel_spmd

    nc = _get_nc()

    def w2(W):
        # [128, 2, 256]: [p, h, f] = W[f, 128h+p]
        return np.ascontiguousarray(
            np.asarray(W, np.float32).T.reshape(2, 128, 256).transpose(1, 0, 2))

    wq2, wk2, wv2 = w2(Wq), w2(Wk), w2(Wv)
    bqs = np.ascontiguousarray(
        (np.asarray(bq, np.float32) / 16.0).reshape(2, 128).T)
    bvr = np.ascontiguousarray(np.asarray(bv, dtype=np.float32))
    masks = _make_masks()
    ebias = _make_ebias()

    in_maps = []
    for c in range(8):
        b, par = c // 2, c % 2
        xb = X[b].reshape(8, 512, E)
        x_shard = np.ascontiguousarray(
            xb[list(CHUNKS[par])].reshape(2048, E))
        in_maps.append({
            "x": x_shard,
            "z": np.ascontiguousarray(Z[b]),
            "wq2": wq2, "wk2": wk2, "wv2": wv2,
            "bqs": bqs, "bvr": bvr,
            "ebias": ebias[par],
            "masks": masks[par],
        })

    res = run_bass_kernel_spmd(nc, in_maps, core_ids=list(range(8)))

    out = np.empty((B, S, E), dtype=np.float32)
    for c in range(8):
        b, par = c // 2, c % 2
        o = res.results[c]["out"].reshape(NSLOT, 512, E)
        for s in range(NSLOT):
            ch = CHUNKS[par][s]
            out[b, 512 * ch:512 * (ch + 1)] = o[s]
    return out


def _numpy_ref(X, Z, mask, Wq, bq, Wk, bk, Wv, bv):
    q = np.einsum("bse,fe->bsf", X, Wq) + bq
    k = np.einsum("bse,fe->bsf", Z, Wk) + bk
    v = np.einsum("bse,fe->bsf", Z, Wv) + bv
    s = np.einsum("bqe,bke->bqk", q, k) / np.sqrt(np.float32(X.shape[-1]))
    s = np.where(mask == 0, -np.inf, s)
    s = s - s.max(axis=-1, keepdims=True)
    p = np.exp(s)
    p /= p.sum(axis=-1, keepdims=True)
    return np.einsum("bqk,bke->bqe", p, v).astype(np.float32)

